# revision 86
# baseline (speedup 1.0000x reference)
"""Causal multi-head attention (16 heads, hd=64) on 8 trn2 NeuronCores.

Sharding: core c -> batch b = c // 4, head-group g = c % 4 (4 heads = 256
columns of Wq/Wk/Wv).  Each core computes its [S, 256] slice of the three
outputs (attn out, K_cache, V_cache); the host gathers slices.

Per-core pipeline (Tile framework), all matmuls in bf16 (f32 PSUM
accumulation, ~4e-3 relative error vs the f32 reference):
  - x and weights arrive partition-major (host pre-arranged) so every
    input DMA descriptor moves a contiguous 4KB run; constants ride the
    scalar queue, x the sync queue, weights the gpsimd queue, so no
    queue's descriptor-gen delays another's critical transfer.
  - KT/QT [c, q] computed directly (lhsT = W chunk), per-partition bias
    added during the DVE eviction; K_cache leaves the chip in kt's [c, s]
    layout (contiguous DMA) and the host transposes it in the gather.
  - Vf natural [s, c] (rank-1 bias matmul) -> V_cache + V_aug tiles
    [k, 65] per head (ones column -> softmax denominator; ones written
    by DVE -- a strided sub-word DMA would RMW-race adjacent columns).
  - scores ST[k, q]: the two heads of a pair run as concurrent
    row-tiled matmuls (K=64, partition offsets 0/64) into one
    [128, 1024] psum tile; diagonal blocks narrowed to the valid q
    range; one exp per k-tile over both heads (ACT, scale=1/8, per-k
    pad bias); the 128-wide partial triangle is zeroed by a DVE multiply
    with a precomputed keep-mask (gpsimd affine_select would thrash the
    gpsimd ucode library against PartitionBroadcast, ~7-13us per swap).
  - AV: out_unnorm[65, q] += V_aug.T @ PT over k-tiles; normalize in
    [d, q] layout: reciprocal of the ones-row sum (fast-approx DVE),
    partition-broadcast (gpsimd DMA, ucode pre-warmed by a dummy
    broadcast during the startup DMA wait), one DVE multiply; `out`
    leaves the chip [c, s] and the host transposes it in the gather,
    like kct.  Pair 1's normalize defers past the next slice's
    projections; the final pair instead broadcasts via a PE rank-1
    outer product (the PE is idle at the tail, and gpsimd's slow
    post-broadcast drain would otherwise stretch the end).
  - Emission order interleaves projections with attention per q-slice so
    attention starts as soon as its k-range is projected; dummy matmuls
    on the early-arriving weights ramp the PE out of its low p-state
    before the first projection.
"""

import numpy as np

P = 128
S = 2048
HIN = 1024
C = 256  # columns per core = 4 heads * 64
HD = 64
NCORES = 8
HC = HIN // P  # 8 contraction chunks
NKT = S // P  # 16 k-tiles
QW = 512  # q-slice width
NQ = S // QW  # 4 q-slices
NPAIR = C // P  # 2 head-pairs per core

_nc_cache = None


def build_nc():
    import concourse.bacc as bacc
    import concourse.mybir as mybir
    from concourse.tile import TileContext
    from contextlib import ExitStack

    f32 = mybir.dt.float32
    f32r = mybir.dt.float32r
    bf16 = mybir.dt.bfloat16
    Exp = mybir.ActivationFunctionType.Exp
    is_ge = mybir.AluOpType.is_ge

    nc = bacc.Bacc(None, target_bir_lowering=False)

    # x and weights arrive partition-major (host pre-arranged) so every
    # DMA descriptor moves a contiguous 4KB run per partition
    xt = nc.declare_dram_parameter("xt", [4, 2, P, (HC // 2) * (S // 4)], bf16,
                                   isOutput=False)
    wq = nc.declare_dram_parameter("wq", [P, HC * C], bf16, isOutput=False)
    wk = nc.declare_dram_parameter("wk", [P, HC * C], bf16, isOutput=False)
    wv = nc.declare_dram_parameter("wv", [P, HC * C], bf16, isOutput=False)
    bqc = nc.declare_dram_parameter("bqc", [P, NPAIR], f32, isOutput=False)
    bkc = nc.declare_dram_parameter("bkc", [P, NPAIR], f32, isOutput=False)
    bv = nc.declare_dram_parameter("bv", [1, C], bf16, isOutput=False)
    padneg = nc.declare_dram_parameter("padneg", [P, NKT], f32, isOutput=False)
    ones = nc.declare_dram_parameter("ones", [P, C], bf16, isOutput=False)
    # causal keep-mask for the diagonal 128-blocks, doubled for the two
    # heads: tri[p, h*P + b] = 1 if b >= p else 0
    tri = nc.declare_dram_parameter("tri", [P, 2 * P], bf16, isOutput=False)
    onesr = nc.declare_dram_parameter("onesr", [1, HD], f32, isOutput=False)
    out = nc.declare_dram_parameter("out", [C, S], bf16, isOutput=True)
    kct = nc.declare_dram_parameter("kct", [C, S], bf16, isOutput=True)
    vc = nc.declare_dram_parameter("vc", [S, C], bf16, isOutput=True)

    with TileContext(nc) as tc, ExitStack() as ctx:
        persist = ctx.enter_context(tc.tile_pool(name="persist", bufs=1))
        xt_sb = persist.tile([P, HC, S], bf16)
        wq_sb = persist.tile([P, HC, C], bf16)
        wk_sb = persist.tile([P, HC, C], bf16)
        wv_sb = persist.tile([P, HC, C], bf16)
        bqc_sb = persist.tile([P, NPAIR], f32)
        bkc_sb = persist.tile([P, NPAIR], f32)
        bv_sb = persist.tile([1, C], bf16)
        pn_sb = persist.tile([P, NKT], f32)
        ones_sb = persist.tile([P, C], bf16)
        tri_sb = persist.tile([P, 2, P], bf16)
        warm_sb = persist.tile([HD, NKT], f32)
        ones_r = persist.tile([1, HD], f32)
        qt_bf = persist.tile([P, NPAIR, S], bf16)
        kt_sb = persist.tile([P, NPAIR, S], bf16)
        va_bf = persist.tile([P, NKT, NPAIR, 2 * (HD + 1)], bf16)

        # small constants on the scalar queue (each dma_start costs
        # ~0.65us of descriptor-gen on its issuing engine, and sync must
        # reach the x pieces as early as possible); none are needed before
        # ~20us into the run
        nc.scalar.dma_start(bqc_sb[:], bqc[:])
        nc.scalar.dma_start(bkc_sb[:], bkc[:])
        nc.scalar.dma_start(bv_sb[:], bv[:])
        nc.scalar.dma_start(pn_sb[:], padneg[:])
        nc.scalar.dma_start(ones_sb[:], ones[:])
        nc.scalar.dma_start(
            tri_sb[:], tri[:].rearrange("p (h b) -> p h b", h=2)
        )
        nc.scalar.dma_start(ones_r[:], onesr[:])
        # weights batched on the gpsimd queue, x in half-quarter pieces on
        # sync, QUARTER-major: the q-slice-0 projections need column
        # 0:512 of ALL 8 chunks, so land every chunk's quarter 0 first
        quarter = S // 4
        half = HC // 2
        for w_sb, w in ((wk_sb, wk), (wq_sb, wq), (wv_sb, wv)):
            nc.gpsimd.dma_start(
                w_sb[:, :, :], w[:, :].rearrange("p (a c) -> p a c", a=HC)
            )
        for h in range(4):
            for g in range(2):
                nc.sync.dma_start(
                    xt_sb[:, g * half : (g + 1) * half,
                          h * quarter : (h + 1) * quarter],
                    xt[h, g, :, :].rearrange("p (a s) -> p a s", a=half),
                )
        # dummy broadcast: loads the gpsimd PartitionBroadcast ucode
        # library (~7us) during the startup DMA wait instead of at the
        # first normalize; with affine_select gone, gpsimd never swaps
        # libraries again
        nc.gpsimd.partition_broadcast(warm_sb[:], pn_sb[0:1, :])
        # ones columns of V_aug (positions 64 and 129).  Written by DVE, not
        # DMA: a strided sub-word DMA write would RMW-race the adjacent
        # DVE-written V columns.
        ones3 = ones_sb[:, : NKT * NPAIR].rearrange("p (a b) -> p a b", a=NKT)
        nc.vector.tensor_copy(
            out=va_bf[:, :, :, HD : HD + 1], in_=ones3[:, :, :, None]
        )
        nc.vector.tensor_copy(
            out=va_bf[:, :, :, 2 * HD + 1 : 2 * HD + 2], in_=ones3[:, :, :, None]
        )

        psum = ctx.enter_context(tc.tile_pool(name="psum", bufs=2, space="PSUM"))
        work = ctx.enter_context(tc.tile_pool(name="work", bufs=3))

        # p-state warm-up: stream dummy matmuls off ones_sb (which lands
        # ~5us in, well before the weights) so the PE reaches full clock by
        # the time the first projection can start; the result is never read
        warm_ps = psum.tile([P, QW], f32, tag="proj", bufs=2, name="warm_ps")
        for r in range(16):
            nc.tensor.matmul(
                warm_ps[:, :C], ones_sb[:, 0:P], ones_sb[:, :],
                start=True, stop=True,
            )

        def kt_qt_slice(qi):
            # for qi=0 the two x chunk-halves are still in flight, so run
            # j 0-3 of both groups of a pair before j 4-7 (PE consumes
            # half 0 while half 1 lands)
            qsl = slice(qi * QW, (qi + 1) * QW)
            for p in range(NPAIR):
                csl = slice(p * P, (p + 1) * P)
                groups = []
                for w_sb, b_sb, dst in (
                    (wk_sb, bkc_sb, kt_sb),
                    (wq_sb, bqc_sb, qt_bf),
                ):
                    ps = psum.tile([P, QW], f32, tag="proj", bufs=2, name="p_ps")
                    groups.append((ps, w_sb, b_sb, dst))
                jws = ((0, 4), (4, 8)) if qi == 0 else ((0, 8),)
                for j0, j1 in jws:
                    for ps, w_sb, b_sb, dst in groups:
                        for j in range(j0, j1):
                            nc.tensor.matmul(
                                ps, w_sb[:, j, csl], xt_sb[:, j, qsl],
                                start=(j == 0), stop=(j == HC - 1),
                            )
                for ps, w_sb, b_sb, dst in groups:
                    nc.vector.tensor_scalar_add(
                        dst[:, p, qsl], ps, b_sb[:, p : p + 1]
                    )

        def v_wave(qi):
            for i in range(4 * qi, 4 * qi + 4):
                ksl = slice(i * P, (i + 1) * P)
                ps = psum.tile([P, QW], f32, tag="proj", bufs=2, name="v_ps")[:, :C]
                for j in range(HC):
                    nc.tensor.matmul(
                        ps, xt_sb[:, j, ksl], wv_sb[:, j, :],
                        start=(j == 0), stop=False,
                    )
                nc.tensor.matmul(
                    ps, ones_sb[:1, :P], bv_sb[:1, :], start=False, stop=True
                )
                sb = work.tile([P, C], bf16, tag="projsb", bufs=4, name="v_sb")
                nc.any.tensor_copy(out=sb[:], in_=ps)
                nc.sync.dma_start(vc[ksl, :], sb[:])
                for p in range(NPAIR):
                    nc.vector.tensor_copy(
                        out=va_bf[:, i, p, 0:HD], in_=sb[:, p * P : p * P + HD]
                    )
                    nc.vector.tensor_copy(
                        out=va_bf[:, i, p, HD + 1 : 2 * HD + 1],
                        in_=sb[:, p * P + HD : (p + 1) * P],
                    )

        def attention_core(qi, p):
            if True:
                av_a = psum.tile([HD + 1, QW], f32, tag="av", bufs=2, name="av_a")
                av_b = psum.tile([HD + 1, QW], f32, tag="av", bufs=2, name="av_b")
                tmax = 4 * qi + 4
                for t in range(tmax):
                    ksl = slice(t * P, (t + 1) * P)
                    d = t - 4 * qi
                    W = QW if d < 0 else QW - d * P
                    q0 = qi * QW + (0 if d < 0 else d * P)
                    st = psum.tile([P, 2 * QW], f32, tag="st", bufs=2, name="st")
                    nc.tensor.matmul(
                        st[:, 0:W], kt_sb[0:HD, p, ksl],
                        qt_bf[0:HD, p, q0 : q0 + W], start=True, stop=True,
                    )
                    nc.tensor.matmul(
                        st[:, QW : QW + W], kt_sb[HD:P, p, ksl],
                        qt_bf[HD:P, p, q0 : q0 + W], start=True, stop=True,
                    )
                    pt = work.tile([P, 2, QW], bf16, tag="pt", bufs=6, name="pt")
                    st3 = st[:].rearrange("p (h w) -> p h w", h=2)[:, :, 0:W]
                    nc.scalar.activation(
                        pt[:, :, 0:W], st3, Exp, bias=pn_sb[:, t : t + 1],
                        scale=0.125,
                    )
                    if d >= 0:
                        nc.vector.tensor_mul(
                            pt[:, :, 0:P], pt[:, :, 0:P], tri_sb[:]
                        )
                    nc.tensor.matmul(
                        av_a[:, QW - W :], va_bf[:, t, p, 0 : HD + 1],
                        pt[:, 0, 0:W], start=(t == 0), stop=(t == tmax - 1),
                    )
                    nc.tensor.matmul(
                        av_b[:, QW - W :], va_bf[:, t, p, HD + 1 : 2 * HD + 2],
                        pt[:, 1, 0:W], start=(t == 0), stop=(t == tmax - 1),
                    )
            return av_a, av_b

        def normalize(qi, p, av_a, av_b, use_pe_outer):
            for h, av in ((0, av_a), (1, av_b)):
                # normalize in [d, q] layout: reciprocal of the ones-row
                # sum, partition-broadcast (gpsimd DMA; PE outer product
                # on the last slice so gpsimd's slow post-broadcast drain
                # doesn't stretch the tail), one DVE multiply; `out`
                # leaves the chip [c, s] (the host transposes it in the
                # gather, like kct)
                dsb = work.tile([1, QW], f32, tag="dsb", bufs=2, name="dsb")
                if use_pe_outer:
                    nc.scalar.copy(dsb[:], av[HD : HD + 1, :])
                else:
                    nc.vector.tensor_copy(out=dsb[:], in_=av[HD : HD + 1, :])
                rcp = work.tile([1, QW], f32, tag="rcp", bufs=2, name="rcp")
                nc.vector.reciprocal_approx_fast(rcp[:], dsb[:])
                rb_sb = work.tile([HD, QW], f32, tag="rb", bufs=2, name="rb")
                if not use_pe_outer:
                    nc.gpsimd.partition_broadcast(rb_sb[:], rcp[:])
                else:
                    # plain-f32 matmul (4 cyc/row, but the PE is idle at the
                    # tail) -- avoids the f32r-rounding cast on the DVE
                    rb_ps = psum.tile([P, 2 * QW], f32, tag="st", bufs=2,
                                      name="rb_ps")[0:HD, 0:QW]
                    nc.tensor.matmul(
                        rb_ps, ones_r[:], rcp[:], start=True, stop=True,
                    )
                    nc.vector.tensor_copy(out=rb_sb[:], in_=rb_ps)
                osb = work.tile([HD, QW], bf16, tag="osb", bufs=3, name="osb")
                nc.vector.tensor_mul(osb[:], av[0:HD, :], rb_sb[:])
                col = p * P + h * HD
                nc.sync.dma_start(
                    out[col : col + HD, qi * QW : (qi + 1) * QW], osb[:]
                )

        # interleaved emission: project a q/k-slice, then run the attention
        # that only needs what's already projected.  Pair 1's normalize is
        # deferred past the next slice's projections, and the very last
        # pair normalizes via the PE outer product (the PE is idle by
        # then) so gpsimd's slow post-broadcast drain starts well before
        # the end.
        pending = None
        for qi in range(NQ):
            kt_qt_slice(qi)
            if pending is not None:
                normalize(qi - 1, 1, *pending, use_pe_outer=False)
            v_wave(qi)
            avs0 = attention_core(qi, 0)
            normalize(qi, 0, *avs0, use_pe_outer=False)
            pending = attention_core(qi, 1)
            # K_cache leaves the chip in kt's [c, s] layout (contiguous
            # DMA); the host transposes it during the gather
            nc.sync.dma_start(
                kct[:, qi * QW : (qi + 1) * QW]
                .rearrange("(a p) s -> p a s", p=P),
                kt_sb[:, :, qi * QW : (qi + 1) * QW],
            )
        normalize(NQ - 1, 1, *pending, use_pe_outer=True)

    nc.finalize()
    return nc


def get_nc():
    global _nc_cache
    if _nc_cache is None:
        _nc_cache = build_nc()
    return _nc_cache


def _w_pre(Wslice):
    # [HIN, C] -> partition-major [P, HC*C]: per partition p a contiguous
    # 4KB run holding its rows of every contraction chunk
    return np.ascontiguousarray(
        Wslice.reshape(HC, P, C).transpose(1, 0, 2).reshape(P, HC * C)
    )


def _x_pre(xT):
    # [HIN, S] -> [4(h), 2(g), P, 4*512]: per (quarter h, chunk-half g,
    # partition p) a contiguous 4KB run
    a = xT.reshape(2, 4, P, 4, QW)  # [g, jm, p, h, s]
    return np.ascontiguousarray(
        a.transpose(3, 0, 2, 1, 4).reshape(4, 2, P, 4 * QW)
    )


def make_in_maps(x, pad_mask, Wq, bq, Wk, bk, Wv, bv):
    from ml_dtypes import bfloat16

    x = np.asarray(x, np.float32)
    pad_mask = np.asarray(pad_mask, np.float32)
    Wq = np.asarray(Wq, bfloat16)
    bq = np.asarray(bq, np.float32)
    Wk = np.asarray(Wk, bfloat16)
    bk = np.asarray(bk, np.float32)
    Wv = np.asarray(Wv, bfloat16)
    bv = np.asarray(bv, bfloat16)
    xts = [_x_pre(x[b].T.astype(bfloat16)) for b in range(2)]
    in_maps = []
    for c in range(NCORES):
        b, g = divmod(c, 4)
        cols = slice(g * C, (g + 1) * C)
        pn = ((pad_mask[b] - 1.0) * 1e6).reshape(NKT, P).T.copy()  # [P, NKT]
        trih = (np.arange(P)[None, :] >= np.arange(P)[:, None]).astype(bfloat16)
        in_maps.append(
            dict(
                xt=xts[b],
                ones=np.ones((P, C), bfloat16),
                tri=np.ascontiguousarray(np.concatenate([trih, trih], axis=1)),
                onesr=np.ones((1, HD), np.float32),
                wq=_w_pre(Wq[:, cols]),
                wk=_w_pre(Wk[:, cols]),
                wv=_w_pre(Wv[:, cols]),
                bqc=np.ascontiguousarray(bq[cols].reshape(NPAIR, P).T),
                bkc=np.ascontiguousarray(bk[cols].reshape(NPAIR, P).T),
                bv=np.ascontiguousarray(bv[cols].reshape(1, C)),
                padneg=pn,
            )
        )
    return in_maps


def gather(results):
    B = 2
    out = np.empty((B, S, HIN), np.float32)
    kcache = np.empty((B, S, HIN), np.float32)
    vcache = np.empty((B, S, HIN), np.float32)
    for c in range(NCORES):
        b, g = divmod(c, 4)
        cols = slice(g * C, (g + 1) * C)
        out[b, :, cols] = results[c]["out"].T
        kcache[b, :, cols] = results[c]["kct"].T
        vcache[b, :, cols] = results[c]["vc"]
    return out, kcache, vcache


def kernel(x, pad_mask, Wq, bq, Wk, bk, Wv, bv):
    from concourse.bass_utils import run_bass_kernel_spmd

    nc = get_nc()
    in_maps = make_in_maps(x, pad_mask, Wq, bq, Wk, bk, Wv, bv)
    res = run_bass_kernel_spmd(nc, in_maps, list(range(NCORES)))
    return gather(res.results)


# revision 87
# speedup vs baseline: 1.2343x; 1.2343x over previous
"""Causal multi-head attention (16 heads, hd=64) on 8 trn2 NeuronCores.

Sharding: core c -> batch b = c // 4, head-group g = c % 4 (4 heads = 256
columns of Wq/Wk/Wv).  Each core computes its [S, 256] slice of the three
outputs (attn out, K_cache, V_cache); the host gathers slices.

Per-core pipeline (Tile framework), all matmuls in bf16 (f32 PSUM
accumulation, ~4e-3 relative error vs the f32 reference):
  - xT [1024, S] is host-transposed x[b]; weights/biases host-sliced.
  - KT/QT [c, q] computed directly (lhsT = W chunk), per-partition bias
    added during the DVE eviction; K_cache leaves the chip in kt's [c, s]
    layout (contiguous DMA) and the host transposes it in the gather.
  - Vf natural [s, c] (rank-1 bias matmul) -> V_cache + V_aug tiles
    [k, 65] per head (ones column -> softmax denominator; ones written
    by DVE -- a strided sub-word DMA would RMW-race adjacent columns).
  - scores ST[k, q]: the two heads of a pair run as concurrent
    row-tiled matmuls (K=64, partition offsets 0/64) into one
    [128, 1024] psum tile; diagonal blocks narrowed to the valid q
    range; one exp per k-tile over both heads (ACT, scale=1/8, per-k
    pad bias), fill-0 affine_select on the 128-wide partial triangle.
  - AV: out_unnorm[65, q] += V_aug.T @ PT over k-tiles; normalize in
    [d, q] layout: reciprocal of the ones-row sum, partition-broadcast
    (gpsimd DMA), one DVE multiply; `out` leaves the chip [c, s] and
    the host transposes it in the gather, like kct.
  - Emission order interleaves projections with attention per q-slice so
    attention starts as soon as its k-range is projected.
"""

import numpy as np

P = 128
S = 2048
HIN = 1024
C = 256  # columns per core = 4 heads * 64
HD = 64
NCORES = 8
HC = HIN // P  # 8 contraction chunks
NKT = S // P  # 16 k-tiles
QW = 512  # q-slice width
NQ = S // QW  # 4 q-slices
NPAIR = C // P  # 2 head-pairs per core

_nc_cache = None


def build_nc():
    import concourse.bacc as bacc
    import concourse.mybir as mybir
    from concourse.tile import TileContext
    from contextlib import ExitStack

    f32 = mybir.dt.float32
    f32r = mybir.dt.float32r
    bf16 = mybir.dt.bfloat16
    Exp = mybir.ActivationFunctionType.Exp
    is_ge = mybir.AluOpType.is_ge

    nc = bacc.Bacc(None, target_bir_lowering=False)

    # x and weights arrive partition-major (host pre-arranged) so every
    # DMA descriptor moves a contiguous 4KB run per partition
    xt = nc.declare_dram_parameter("xt", [4, 2, P, (HC // 2) * (S // 4)], bf16,
                                   isOutput=False)
    wq = nc.declare_dram_parameter("wq", [P, HC * C], bf16, isOutput=False)
    wk = nc.declare_dram_parameter("wk", [P, HC * C], bf16, isOutput=False)
    wv = nc.declare_dram_parameter("wv", [P, HC * C], bf16, isOutput=False)
    bqc = nc.declare_dram_parameter("bqc", [P, NPAIR], f32, isOutput=False)
    bkc = nc.declare_dram_parameter("bkc", [P, NPAIR], f32, isOutput=False)
    bv = nc.declare_dram_parameter("bv", [1, C], bf16, isOutput=False)
    padneg = nc.declare_dram_parameter("padneg", [P, NKT], f32, isOutput=False)
    ones = nc.declare_dram_parameter("ones", [P, C], bf16, isOutput=False)
    # causal keep-mask for the diagonal 128-blocks, doubled for the two
    # heads: tri[p, h*P + b] = 1 if b >= p else 0
    tri = nc.declare_dram_parameter("tri", [P, 2 * P], bf16, isOutput=False)
    onesr = nc.declare_dram_parameter("onesr", [1, HD], f32r, isOutput=False)
    out = nc.declare_dram_parameter("out", [C, S], bf16, isOutput=True)
    kct = nc.declare_dram_parameter("kct", [C, S], bf16, isOutput=True)
    vc = nc.declare_dram_parameter("vc", [S, C], bf16, isOutput=True)

    with TileContext(nc) as tc, ExitStack() as ctx:
        persist = ctx.enter_context(tc.tile_pool(name="persist", bufs=1))
        xt_sb = persist.tile([P, HC, S], bf16)
        wq_sb = persist.tile([P, HC, C], bf16)
        wk_sb = persist.tile([P, HC, C], bf16)
        wv_sb = persist.tile([P, HC, C], bf16)
        bqc_sb = persist.tile([P, NPAIR], f32)
        bkc_sb = persist.tile([P, NPAIR], f32)
        bv_sb = persist.tile([1, C], bf16)
        pn_sb = persist.tile([P, NKT], f32)
        ones_sb = persist.tile([P, C], bf16)
        tri_sb = persist.tile([P, 2, P], bf16)
        warm_sb = persist.tile([HD, NKT], f32)
        ones_r = persist.tile([1, HD], f32r)
        qt_bf = persist.tile([P, NPAIR, S], bf16)
        kt_sb = persist.tile([P, NPAIR, S], bf16)
        va_bf = persist.tile([P, NKT, NPAIR, 2 * (HD + 1)], bf16)

        # small constants on the scalar queue (each dma_start costs
        # ~0.65us of descriptor-gen on its issuing engine, and sync must
        # reach the x pieces as early as possible); none are needed before
        # ~20us into the run
        nc.scalar.dma_start(bqc_sb[:], bqc[:])
        nc.scalar.dma_start(bkc_sb[:], bkc[:])
        nc.scalar.dma_start(bv_sb[:], bv[:])
        nc.scalar.dma_start(pn_sb[:], padneg[:])
        nc.scalar.dma_start(ones_sb[:], ones[:])
        nc.scalar.dma_start(
            tri_sb[:], tri[:].rearrange("p (h b) -> p h b", h=2)
        )
        nc.scalar.dma_start(ones_r[:], onesr[:])
        # weights batched on the gpsimd queue, x in half-quarter pieces on
        # sync, QUARTER-major: the q-slice-0 projections need column
        # 0:512 of ALL 8 chunks, so land every chunk's quarter 0 first
        quarter = S // 4
        half = HC // 2
        for w_sb, w in ((wk_sb, wk), (wq_sb, wq), (wv_sb, wv)):
            nc.gpsimd.dma_start(
                w_sb[:, :, :], w[:, :].rearrange("p (a c) -> p a c", a=HC)
            )
        for h in range(4):
            for g in range(2):
                nc.sync.dma_start(
                    xt_sb[:, g * half : (g + 1) * half,
                          h * quarter : (h + 1) * quarter],
                    xt[h, g, :, :].rearrange("p (a s) -> p a s", a=half),
                )
        # dummy broadcast: loads the gpsimd PartitionBroadcast ucode
        # library (~7us) during the startup DMA wait instead of at the
        # first normalize; with affine_select gone, gpsimd never swaps
        # libraries again
        nc.gpsimd.partition_broadcast(warm_sb[:], pn_sb[0:1, :])
        # ones columns of V_aug (positions 64 and 129).  Written by DVE, not
        # DMA: a strided sub-word DMA write would RMW-race the adjacent
        # DVE-written V columns.
        ones3 = ones_sb[:, : NKT * NPAIR].rearrange("p (a b) -> p a b", a=NKT)
        nc.vector.tensor_copy(
            out=va_bf[:, :, :, HD : HD + 1], in_=ones3[:, :, :, None]
        )
        nc.vector.tensor_copy(
            out=va_bf[:, :, :, 2 * HD + 1 : 2 * HD + 2], in_=ones3[:, :, :, None]
        )

        psum = ctx.enter_context(tc.tile_pool(name="psum", bufs=2, space="PSUM"))
        work = ctx.enter_context(tc.tile_pool(name="work", bufs=3))

        # p-state warm-up: stream dummy matmuls off the weights (which
        # land ~3us before x) so the PE reaches full clock by the time the
        # first projection can start; the result is never read
        warm_ps = psum.tile([P, QW], f32, tag="proj", bufs=2, name="warm_ps")
        for r in range(7):
            nc.tensor.matmul(
                warm_ps, wk_sb[:, 0, 0:P],
                wk_sb[:, 2 * (r % 4) : 2 * (r % 4) + 2, :],
                start=True, stop=True,
            )

        def kt_qt_slice(qi):
            # for qi=0 the two x chunk-halves are still in flight, so run
            # j 0-3 of both groups of a pair before j 4-7 (PE consumes
            # half 0 while half 1 lands)
            qsl = slice(qi * QW, (qi + 1) * QW)
            for p in range(NPAIR):
                csl = slice(p * P, (p + 1) * P)
                groups = []
                for w_sb, b_sb, dst in (
                    (wk_sb, bkc_sb, kt_sb),
                    (wq_sb, bqc_sb, qt_bf),
                ):
                    ps = psum.tile([P, QW], f32, tag="proj", bufs=2, name="p_ps")
                    groups.append((ps, w_sb, b_sb, dst))
                jws = ((0, 4), (4, 8)) if qi == 0 else ((0, 8),)
                for j0, j1 in jws:
                    for ps, w_sb, b_sb, dst in groups:
                        for j in range(j0, j1):
                            nc.tensor.matmul(
                                ps, w_sb[:, j, csl], xt_sb[:, j, qsl],
                                start=(j == 0), stop=(j == HC - 1),
                            )
                for ps, w_sb, b_sb, dst in groups:
                    nc.vector.tensor_scalar_add(
                        dst[:, p, qsl], ps, b_sb[:, p : p + 1]
                    )

        def v_wave(qi):
            for i in range(4 * qi, 4 * qi + 4):
                ksl = slice(i * P, (i + 1) * P)
                ps = psum.tile([P, QW], f32, tag="proj", bufs=2, name="v_ps")[:, :C]
                for j in range(HC):
                    nc.tensor.matmul(
                        ps, xt_sb[:, j, ksl], wv_sb[:, j, :],
                        start=(j == 0), stop=False,
                    )
                nc.tensor.matmul(
                    ps, ones_sb[:1, :P], bv_sb[:1, :], start=False, stop=True
                )
                sb = work.tile([P, C], bf16, tag="projsb", bufs=4, name="v_sb")
                nc.any.tensor_copy(out=sb[:], in_=ps)
                nc.sync.dma_start(vc[ksl, :], sb[:])
                for p in range(NPAIR):
                    nc.vector.tensor_copy(
                        out=va_bf[:, i, p, 0:HD], in_=sb[:, p * P : p * P + HD]
                    )
                    nc.vector.tensor_copy(
                        out=va_bf[:, i, p, HD + 1 : 2 * HD + 1],
                        in_=sb[:, p * P + HD : (p + 1) * P],
                    )

        def attention_core(qi, p):
            if True:
                av_a = psum.tile([HD + 1, QW], f32, tag="av", bufs=2, name="av_a")
                av_b = psum.tile([HD + 1, QW], f32, tag="av", bufs=2, name="av_b")
                tmax = 4 * qi + 4
                for t in range(tmax):
                    ksl = slice(t * P, (t + 1) * P)
                    d = t - 4 * qi
                    W = QW if d < 0 else QW - d * P
                    q0 = qi * QW + (0 if d < 0 else d * P)
                    st = psum.tile([P, 2 * QW], f32, tag="st", bufs=2, name="st")
                    nc.tensor.matmul(
                        st[:, 0:W], kt_sb[0:HD, p, ksl],
                        qt_bf[0:HD, p, q0 : q0 + W], start=True, stop=True,
                    )
                    nc.tensor.matmul(
                        st[:, QW : QW + W], kt_sb[HD:P, p, ksl],
                        qt_bf[HD:P, p, q0 : q0 + W], start=True, stop=True,
                    )
                    pt = work.tile([P, 2, QW], bf16, tag="pt", bufs=6, name="pt")
                    st3 = st[:].rearrange("p (h w) -> p h w", h=2)[:, :, 0:W]
                    nc.scalar.activation(
                        pt[:, :, 0:W], st3, Exp, bias=pn_sb[:, t : t + 1],
                        scale=0.125,
                    )
                    if d >= 0:
                        nc.vector.tensor_mul(
                            pt[:, :, 0:P], pt[:, :, 0:P], tri_sb[:]
                        )
                    nc.tensor.matmul(
                        av_a[:, QW - W :], va_bf[:, t, p, 0 : HD + 1],
                        pt[:, 0, 0:W], start=(t == 0), stop=(t == tmax - 1),
                    )
                    nc.tensor.matmul(
                        av_b[:, QW - W :], va_bf[:, t, p, HD + 1 : 2 * HD + 2],
                        pt[:, 1, 0:W], start=(t == 0), stop=(t == tmax - 1),
                    )
            return av_a, av_b

        def normalize(qi, p, av_a, av_b, use_pe_outer):
            for h, av in ((0, av_a), (1, av_b)):
                # normalize in [d, q] layout: reciprocal of the ones-row
                # sum, partition-broadcast (gpsimd DMA; PE outer product
                # on the last slice so gpsimd's slow post-broadcast drain
                # doesn't stretch the tail), one DVE multiply; `out`
                # leaves the chip [c, s] (the host transposes it in the
                # gather, like kct)
                dsb = work.tile([1, QW], f32, tag="dsb", bufs=2, name="dsb")
                if use_pe_outer:
                    nc.scalar.copy(dsb[:], av[HD : HD + 1, :])
                else:
                    nc.vector.tensor_copy(out=dsb[:], in_=av[HD : HD + 1, :])
                rcp = work.tile([1, QW], f32, tag="rcp", bufs=2, name="rcp")
                nc.vector.reciprocal_approx_fast(rcp[:], dsb[:])
                rb_sb = work.tile([HD, QW], f32, tag="rb", bufs=2, name="rb")
                if not use_pe_outer:
                    nc.gpsimd.partition_broadcast(rb_sb[:], rcp[:])
                else:
                    rcp_r = work.tile([1, QW], f32r, tag="rcpr", bufs=2,
                                      name="rcpr")
                    nc.vector.tensor_copy(out=rcp_r[:], in_=rcp[:])
                    rb_ps = psum.tile([P, 2 * QW], f32, tag="st", bufs=2,
                                      name="rb_ps")[0:HD, 0:QW]
                    nc.tensor.matmul(
                        rb_ps, ones_r[:], rcp_r[:], start=True, stop=True,
                    )
                    nc.vector.tensor_copy(out=rb_sb[:], in_=rb_ps)
                osb = work.tile([HD, QW], bf16, tag="osb", bufs=3, name="osb")
                nc.vector.tensor_mul(osb[:], av[0:HD, :], rb_sb[:])
                col = p * P + h * HD
                nc.sync.dma_start(
                    out[col : col + HD, qi * QW : (qi + 1) * QW], osb[:]
                )

        # interleaved emission: project a q/k-slice, then run the attention
        # that only needs what's already projected.  Pair 1's normalize is
        # deferred past the next slice's projections, and the very last
        # pair normalizes via the PE outer product (the PE is idle by
        # then) so gpsimd's slow post-broadcast drain starts well before
        # the end.
        pending = None
        for qi in range(NQ):
            kt_qt_slice(qi)
            if pending is not None:
                normalize(qi - 1, 1, *pending, use_pe_outer=False)
            v_wave(qi)
            avs0 = attention_core(qi, 0)
            normalize(qi, 0, *avs0, use_pe_outer=False)
            pending = attention_core(qi, 1)
            # K_cache leaves the chip in kt's [c, s] layout (contiguous
            # DMA); the host transposes it during the gather
            nc.sync.dma_start(
                kct[:, qi * QW : (qi + 1) * QW]
                .rearrange("(a p) s -> p a s", p=P),
                kt_sb[:, :, qi * QW : (qi + 1) * QW],
            )
        normalize(NQ - 1, 1, *pending, use_pe_outer=True)

    nc.finalize()
    return nc


def get_nc():
    global _nc_cache
    if _nc_cache is None:
        _nc_cache = build_nc()
    return _nc_cache


def _w_pre(Wslice):
    # [HIN, C] -> partition-major [P, HC*C]: per partition p a contiguous
    # 4KB run holding its rows of every contraction chunk
    return np.ascontiguousarray(
        Wslice.reshape(HC, P, C).transpose(1, 0, 2).reshape(P, HC * C)
    )


def _x_pre(xT):
    # [HIN, S] -> [4(h), 2(g), P, 4*512]: per (quarter h, chunk-half g,
    # partition p) a contiguous 4KB run
    a = xT.reshape(2, 4, P, 4, QW)  # [g, jm, p, h, s]
    return np.ascontiguousarray(
        a.transpose(3, 0, 2, 1, 4).reshape(4, 2, P, 4 * QW)
    )


def make_in_maps(x, pad_mask, Wq, bq, Wk, bk, Wv, bv):
    from ml_dtypes import bfloat16

    x = np.asarray(x, np.float32)
    pad_mask = np.asarray(pad_mask, np.float32)
    Wq = np.asarray(Wq, bfloat16)
    bq = np.asarray(bq, np.float32)
    Wk = np.asarray(Wk, bfloat16)
    bk = np.asarray(bk, np.float32)
    Wv = np.asarray(Wv, bfloat16)
    bv = np.asarray(bv, bfloat16)
    xts = [_x_pre(x[b].T.astype(bfloat16)) for b in range(2)]
    in_maps = []
    for c in range(NCORES):
        b, g = divmod(c, 4)
        cols = slice(g * C, (g + 1) * C)
        pn = ((pad_mask[b] - 1.0) * 1e6).reshape(NKT, P).T.copy()  # [P, NKT]
        trih = (np.arange(P)[None, :] >= np.arange(P)[:, None]).astype(bfloat16)
        in_maps.append(
            dict(
                xt=xts[b],
                ones=np.ones((P, C), bfloat16),
                tri=np.ascontiguousarray(np.concatenate([trih, trih], axis=1)),
                onesr=np.ones((1, HD), np.float32),
                wq=_w_pre(Wq[:, cols]),
                wk=_w_pre(Wk[:, cols]),
                wv=_w_pre(Wv[:, cols]),
                bqc=np.ascontiguousarray(bq[cols].reshape(NPAIR, P).T),
                bkc=np.ascontiguousarray(bk[cols].reshape(NPAIR, P).T),
                bv=np.ascontiguousarray(bv[cols].reshape(1, C)),
                padneg=pn,
            )
        )
    return in_maps


def gather(results):
    B = 2
    out = np.empty((B, S, HIN), np.float32)
    kcache = np.empty((B, S, HIN), np.float32)
    vcache = np.empty((B, S, HIN), np.float32)
    for c in range(NCORES):
        b, g = divmod(c, 4)
        cols = slice(g * C, (g + 1) * C)
        out[b, :, cols] = results[c]["out"].T
        kcache[b, :, cols] = results[c]["kct"].T
        vcache[b, :, cols] = results[c]["vc"]
    return out, kcache, vcache


def kernel(x, pad_mask, Wq, bq, Wk, bk, Wv, bv):
    from concourse.bass_utils import run_bass_kernel_spmd

    nc = get_nc()
    in_maps = make_in_maps(x, pad_mask, Wq, bq, Wk, bk, Wv, bv)
    res = run_bass_kernel_spmd(nc, in_maps, list(range(NCORES)))
    return gather(res.results)
